# revision 2
# baseline (speedup 1.0000x reference)
"""CLIP contrastive loss on 8 Trainium2 NeuronCores.

Math (reference): with n = 4096, 2n = 8192 rows of L2-normalized features,
  logits_per_image = scale * img[:n] @ txt.T        [n, 2n]
  logits_per_text  = scale * txt[:n] @ img.T        [n, 2n]
  loss = (ce(logits_per_image) + ce(logits_per_text)) / 2,
  ce(L) = mean_r(logsumexp(L[r]) - L[r, r]).

Both CE terms are slices of the single matrix M = scale * img @ txt.T:
logits_per_image = M[:n, :] and logits_per_text = M[:, :n].T, so the
shared block M[:n, :n] is computed ONCE (3n^2 dot products instead of 4n^2):

  region T = M[:n, :]     -> row sums of exp give the image-CE denominators;
                             exp of the first n columns is also kept (bf16)
                             for the text-CE column sums.
  region B = M[n:, :n]    -> exp kept for the text-CE column sums.

Distribution: core c owns rows [c*512, (c+1)*512) of BOTH regions.  Matmuls
run in fp8(e4m3) DoubleRow perf mode (78.6 GMAC/s/core = the PE roofline;
per-core MM work is 41.5us and every other engine is budgeted under it).

Engine split (the v1 kernel was ACT-bound at ~54us of exp work):
  ACT  does only the T-region exp, fused with the row sums (accum_out):
       20 ACTIVATEs + READ_ACCUMULATORs ~= 40us.
  DVE  does the B-region exp with a Schraudolph bit-trick (exp(x) ~=
       bitcast_bf16(round(x*128*log2(e) + 16248.9)) -- the int16 add puts
       floor in the bf16 exponent field and the linear remainder in the
       mantissa; the constant centers E[ln(approx/true)] at 0, worst-case
       per-element error +-4.5% which averages out in the 8192-row column
       sums; final-loss contribution < 1e-3 absolute) plus the column-sum
       add trees ~= 35us.
  PE   interleaves T and B m-blocks (B one chunk behind T) so the two psum
       consumers (ACT for T tiles, DVE for B tiles) drain alternating psum
       buffers concurrently and neither paces the matmul stream.

The schedule ends on the row-sum-only T tail chunks so nothing but one
ACTIVATE + tiny row-sum DMAs trails the last matmul.  The eight [128, cw]
exp tiles per shared chunk (4 T + 4 B m-blocks) are tree-added on DVE into
comb[128, 4096] (DMA'd out mid-kernel); the host finishes the 128-partition
column reduction, the diagonal (exact, f64), and the final mean.

Features are quantized host-side: stationary = e4m3(img * scale/16),
moving = e4m3(txt * 16), so psum = scale * <img, txt> directly.  The moving
tensor is staged in DRAM as one tensor PER CHUNK shaped like its SBUF tile,
so every partition's chunk data is a single contiguous KC*cw-byte run and
the DMA moves ~8 KB packets.
"""

import numpy as np
import ml_dtypes

import concourse.tile as tile
from concourse import bacc, mybir
from concourse.bass_utils import run_bass_kernel_spmd

TWO_N = 8192   # total rows (and T logits columns)
N = 4096       # CE rows
D = 512        # embedding dim
C = 8          # cores
R = N // C     # CE rows per core = 512
KC = D // 128  # contraction k-tiles = 4
W = 2048       # widest column chunk (psum tile width)
MB = R // 128  # 128-row blocks per core = 4
QSCALE = 16.0  # feature pre-scale before e4m3 quantization

# Schraudolph exp2 bit-trick constants (bf16 target):
#   i16 = round(x * 128*log2(e) + EXP_B); bitcast(i16) ~= exp(x).
# EXP_B = 127*128 - 7.33: the -7.33 makes E[ln(approx/true)] = 0 over a
# uniform mantissa-fraction distribution (the +4%/-4.5% piecewise-linear
# error then cancels in large sums).
EXP_A = 184.66496523378733   # 128 * log2(e)
EXP_B = 16248.9

BF16 = mybir.dt.bfloat16
FP8 = mybir.dt.float8e4
F32 = mybir.dt.float32
I16 = mybir.dt.int16

_CACHE = {}

# T-region column chunks: the first n=4096 columns (shared with B) as
# [512, 1536, 2048] (the 512 lead-in starts the consumers early and is on
# the ring first so the PE starts ~2us sooner), then the row-only tail half
# as [2048, 2048].
SHARED_CHUNKS = [512, 1536, 2048]
TAIL_CHUNKS = [2048, 2048]
T_CHUNKS = SHARED_CHUNKS + TAIL_CHUNKS
NSH = len(SHARED_CHUNKS)
PCOLS = len(T_CHUNKS)  # partials free columns (T row-sum pieces per m)


def _build():
    """Build the (core-uniform) Bass/Tile program once."""
    nc = bacc.Bacc("TRN2", target_bir_lowering=False, debug=False, num_devices=C)

    # stat1 split: the m0 slice is its own (contiguous) tensor so a tiny
    # 32KB DMA on the sync ring unblocks the very first matmul ~2us before
    # the full stationary lands on the slower gpsimd ring
    stat1a = nc.dram_tensor("stat1a", [128, KC, 128], FP8, kind="ExternalInput").ap()
    stat1b = nc.dram_tensor("stat1b", [128, KC, R - 128], FP8, kind="ExternalInput").ap()
    stat2 = nc.dram_tensor("stat2", [128, KC, R], FP8, kind="ExternalInput").ap()
    movs = [
        nc.dram_tensor(f"mov{ci}", [128, KC, cw], FP8, kind="ExternalInput").ap()
        for ci, cw in enumerate(T_CHUNKS)
    ]
    out = nc.dram_tensor("out", [128, MB, PCOLS], F32, kind="ExternalOutput").ap()
    outc = nc.dram_tensor("outc", [128, N], BF16, kind="ExternalOutput").ap()

    DR = mybir.MatmulPerfMode.DoubleRow
    EXP = mybir.ActivationFunctionType.Exp
    add = mybir.AluOpType.add
    mult = mybir.AluOpType.mult

    t_off = [0]
    for cw in T_CHUNKS:
        t_off.append(t_off[-1] + cw)

    with tile.TileContext(nc) as tc:
        with (
            tc.tile_pool(name="stat", bufs=1) as stat_pool,
            tc.tile_pool(name="acc", bufs=1) as acc_pool,
            tc.tile_pool(name="mov", bufs=1) as mov_pool,
            tc.tile_pool(name="exp", bufs=1) as exp_pool,
            tc.tile_pool(name="red", bufs=1) as red_pool,
            tc.tile_pool(name="psum", bufs=2, space="PSUM") as psum_pool,
        ):
            st1a = stat_pool.tile([128, KC, 128], FP8, tag="st1a")
            st1b = stat_pool.tile([128, KC, R - 128], FP8, tag="st1b")
            st2 = stat_pool.tile([128, KC, R], FP8, tag="st2")

            # PE warm-up: throwaway matmuls with no DMA deps start the
            # HAM clock ramp (1.2 -> 2.4 GHz) while the first chunk streams.
            warm = stat_pool.tile([128, 512], BF16, tag="warm")
            nc.vector.memset(warm[:], 0.0)
            wps = psum_pool.tile([128, W], F32, tag="ps")
            for _ in range(6):
                nc.tensor.matmul(
                    wps[:, 0:512], warm[:, 0:128], warm[:, 0:512],
                    start=True, stop=True,
                )

            # partials[p, m, i] = sum_j exp(T[m-block row p, chunk i cols j])
            partials = acc_pool.tile([128, MB, PCOLS], F32, tag="partials")
            nc.vector.memset(partials[:], 0.0)

            # st1's m0 slice goes FIRST on the sync ring (ahead of chunk0);
            # stat1b and stat2 ride the gpsimd ring (stat1b first: it is
            # needed at T(c0,m1) ~1us after the first matmul; stat2 is not
            # needed until the B(c0) tiles a chunk later)
            nc.sync.dma_start(st1a[:], stat1a[:])
            nc.gpsimd.dma_start(st1b[:], stat1b[:])
            nc.gpsimd.dma_start(st2[:], stat2[:])

            mov_tiles = {}   # ci -> (tile, cw) for shared chunks (kept for B)
            exp_t = {}       # (region, ci, m) -> exp tile (bf16 / int16)

            def mm_block(ps, st, blk, mt, cw):
                # st/blk: stationary tile and its local 128-row block index
                for so in range(0, cw, 512):
                    sw = min(512, cw - so)
                    for k in (0, 2):
                        nc.tensor.matmul(
                            ps[:, so:so + sw],
                            st[:, k:k + 2, blk * 128:(blk + 1) * 128],
                            mt[:, k:k + 2, so:so + sw],
                            start=(k == 0),
                            stop=(k == 2),
                            perf_mode=DR,
                        )

            def st1_of(m):
                return (st1a, 0) if m == 0 else (st1b, m - 1)

            def t_tile(ci, m):
                """T-region psum tile (ci, m): matmuls + ACT exp/row-sum."""
                cw = T_CHUNKS[ci]
                ps = psum_pool.tile([128, W], F32, tag="ps")
                stm, blk = st1_of(m)
                mm_block(ps, stm, blk, mov_tiles[ci][0], cw)
                if ci < NSH:
                    xt = exp_pool.tile([128, cw], BF16, tag=f"xT{ci}_{m}")
                    exp_t[("T", ci, m)] = xt
                    out_ap = xt[:, 0:cw]
                else:
                    out_ap = ps[:, 0:cw]   # tail: exp in place, rowsum only
                nc.scalar.activation(
                    out_ap, ps[:, 0:cw], EXP, bias=0.0,
                    accum_out=partials[:, m, ci:ci + 1],
                )

            def b_tile(ci, m):
                """B-region psum tile (ci, m): matmuls + DVE bit-trick exp."""
                cw = T_CHUNKS[ci]
                ps = psum_pool.tile([128, W], F32, tag="ps")
                mm_block(ps, st2, m, mov_tiles[ci][0], cw)
                xb = exp_pool.tile([128, cw], I16, tag=f"xB{ci}_{m}")
                exp_t[("B", ci, m)] = xb
                nc.vector.tensor_scalar(
                    xb[:, 0:cw], ps[:, 0:cw], EXP_A, EXP_B, op0=mult, op1=add,
                )

            def trees(ci):
                """Column-sum add trees for shared chunk ci -> comb -> DMA."""
                cw = T_CHUNKS[ci]
                t0, t1, t2, t3 = (exp_t[("T", ci, m)] for m in range(MB))
                b0, b1, b2, b3 = (
                    exp_t[("B", ci, m)][:].bitcast(BF16) for m in range(MB)
                )
                a01 = red_pool.tile([128, cw], BF16, tag=f"a01_{ci}")
                a23 = red_pool.tile([128, cw], BF16, tag=f"a23_{ci}")
                b01 = red_pool.tile([128, cw], BF16, tag=f"b01_{ci}")
                b23 = red_pool.tile([128, cw], BF16, tag=f"b23_{ci}")
                comb = red_pool.tile([128, cw], BF16, tag=f"cb_{ci}")
                nc.vector.tensor_tensor(out=a01[:], in0=t0[:], in1=t1[:], op=add)
                nc.vector.tensor_tensor(out=a23[:], in0=t2[:], in1=t3[:], op=add)
                nc.vector.tensor_tensor(out=b01[:], in0=b0, in1=b1, op=add)
                nc.vector.tensor_tensor(out=b23[:], in0=b2, in1=b3, op=add)
                nc.vector.tensor_tensor(out=a01[:], in0=a01[:], in1=a23[:], op=add)
                nc.vector.tensor_tensor(out=b01[:], in0=b01[:], in1=b23[:], op=add)
                nc.vector.tensor_tensor(out=comb[:], in0=a01[:], in1=b01[:], op=add)
                nc.gpsimd.dma_start(outc[:, t_off[ci]:t_off[ci] + cw], comb[:])

            def load_mov(ci):
                cw = T_CHUNKS[ci]
                mt = mov_pool.tile([128, KC, cw], FP8, tag=f"ms{ci}")
                mov_tiles[ci] = (mt, cw)
                # all mov chunks stream FIFO on the sync ring (~190 GB/s,
                # stays ahead of the PE; splitting across rings regressed)
                nc.sync.dma_start(mt[:, :, 0:cw], movs[ci][:])

            for ci in range(PCOLS):
                load_mov(ci)

            # Schedule: B runs exactly one chunk behind T, interleaved at
            # m-block granularity, so consecutive psum tiles are drained by
            # different engines (ACT for T, DVE for B).  Trees for chunk c
            # are emitted two chunks later so they never delay a psum drain.
            for m in range(MB):
                t_tile(0, m)
            for m in range(MB):
                t_tile(1, m)
                b_tile(0, m)
            for m in range(MB):
                t_tile(2, m)
                b_tile(1, m)
                if m == 1:
                    trees(0)
            for m in range(MB):
                t_tile(3, m)
                b_tile(2, m)
                if m == 1:
                    trees(1)
            for m in range(MB):
                t_tile(4, m)
                if m == 0:
                    trees(2)

            # T row-sum partials out on the sync ring (tiny, 80B/partition)
            nc.sync.dma_start(out[:], partials[:])

    nc.compile()
    return nc


def _get_nc():
    if "nc" not in _CACHE:
        _CACHE["nc"] = _build()
    return _CACHE["nc"]


def _prep_inputs(image_features, text_features, logit_scale):
    img = np.asarray(image_features, dtype=np.float32)
    txt = np.asarray(text_features, dtype=np.float32)
    scale = float(np.asarray(logit_scale, dtype=np.float32))

    # mov{ci}[p, k, j] = QSCALE * txt[t_off[ci] + j, k*128 + p], e4m3
    a = np.ascontiguousarray(txt.T * np.float32(QSCALE)).reshape(KC, 128, TWO_N)
    movf = a.transpose(1, 0, 2).astype(ml_dtypes.float8_e4m3)  # [128, KC, 2N]
    mov_chunks = {}
    off = 0
    for ci, cw in enumerate(T_CHUNKS):
        mov_chunks[f"mov{ci}"] = np.ascontiguousarray(movf[:, :, off:off + cw])
        off += cw

    def stat_layout(rows):
        # [p, k, m] = (scale/QSCALE) * feat[row0 + m, k*128 + p], e4m3
        a = (rows * np.float32(scale / QSCALE)).T.reshape(KC, 128, R)
        return np.ascontiguousarray(a.transpose(1, 0, 2).astype(ml_dtypes.float8_e4m3))

    def stat1_split(c):
        full = stat_layout(img[c * R:(c + 1) * R])
        return (np.ascontiguousarray(full[:, :, 0:128]),
                np.ascontiguousarray(full[:, :, 128:R]))

    in_maps = []
    for c in range(C):
        s1a, s1b = stat1_split(c)
        in_maps.append({
            "stat1a": s1a,
            "stat1b": s1b,
            "stat2": stat_layout(img[N + c * R:N + (c + 1) * R]),
            **mov_chunks,
        })
    # diagonal logits (same for both CE terms): scale * <img_r, txt_r>
    diag = scale * np.sum(
        img[:N].astype(np.float64) * txt[:N].astype(np.float64), axis=1
    )
    return in_maps, diag


def _finish(results, diag):
    # image CE: S_img[c*R + m*128 + p] = sum_i out[c][p, m, i]
    s = np.stack([results[c]["out"] for c in range(C)]).astype(np.float64)
    s_img = s.sum(axis=-1)  # [c, p, m]
    rows = (
        np.arange(C)[:, None, None] * R
        + np.arange(MB)[None, None, :] * 128
        + np.arange(128)[None, :, None]
    )  # [c, p, m]
    ce_img = np.mean(np.log(s_img) - diag[rows])
    # text CE: S_txt[j] = sum_c sum_p comb[c][p, j]
    combs = np.stack([np.asarray(results[c]["outc"], dtype=np.float64) for c in range(C)])
    s_txt = combs.sum(axis=(0, 1))  # [N]
    ce_txt = np.mean(np.log(s_txt) - diag)
    return np.float32((ce_img + ce_txt) / 2.0)


def kernel(image_features, text_features, logit_scale):
    nc = _get_nc()
    in_maps, diag = _prep_inputs(image_features, text_features, logit_scale)
    res = run_bass_kernel_spmd(nc, in_maps, list(range(C)))
    return _finish(res.results, diag)


if __name__ == "__main__":
    rng = np.random.default_rng(0)
    img = rng.standard_normal((TWO_N, D), dtype=np.float32)
    txt = rng.standard_normal((TWO_N, D), dtype=np.float32)
    img /= np.linalg.norm(img, axis=-1, keepdims=True)
    txt /= np.linalg.norm(txt, axis=-1, keepdims=True)
    print(kernel(img, txt, np.float32(100.0)))


# revision 6
# speedup vs baseline: 1.0561x; 1.0561x over previous
"""CLIP contrastive loss on 8 Trainium2 NeuronCores.

Math (reference): with n = 4096, 2n = 8192 rows of L2-normalized features,
  logits_per_image = scale * img[:n] @ txt.T        [n, 2n]
  logits_per_text  = scale * txt[:n] @ img.T        [n, 2n]
  loss = (ce(logits_per_image) + ce(logits_per_text)) / 2,
  ce(L) = mean_r(logsumexp(L[r]) - L[r, r]).

Both CE terms are slices of the single matrix M = scale * img @ txt.T:
logits_per_image = M[:n, :] and logits_per_text = M[:, :n].T, so the
shared block M[:n, :n] is computed ONCE (3n^2 dot products instead of 4n^2):

  region T = M[:n, :]     -> row sums of exp give the image-CE denominators;
                             exp of the first n columns is also kept (bf16)
                             for the text-CE column sums.
  region B = M[n:, :n]    -> exp kept for the text-CE column sums.

Distribution: core c owns rows [c*512, (c+1)*512) of BOTH regions.  Matmuls
run in fp8(e4m3) DoubleRow perf mode (78.6 GMAC/s/core = the PE roofline;
per-core MM work is 41.5us and every other engine is budgeted under it).

Engine split (the v1 kernel was ACT-bound at ~54us of exp work):
  ACT  does only the T-region exp, fused with the row sums (accum_out):
       20 ACTIVATEs + READ_ACCUMULATORs ~= 40us.
  DVE  does the B-region exp with a Schraudolph bit-trick (exp(x) ~=
       bitcast_bf16(round(x*128*log2(e) + 16248.9)) -- the int16 add puts
       floor in the bf16 exponent field and the linear remainder in the
       mantissa; the constant centers E[ln(approx/true)] at 0, worst-case
       per-element error +-4.5% which averages out in the 8192-row column
       sums; final-loss contribution < 1e-3 absolute) plus the column-sum
       add trees ~= 35us.
  PE   interleaves T and B m-blocks (B one chunk behind T) so the two psum
       consumers (ACT for T tiles, DVE for B tiles) drain alternating psum
       buffers concurrently and neither paces the matmul stream.

The schedule ends on the row-sum-only T tail chunks so nothing but one
ACTIVATE + tiny row-sum DMAs trails the last matmul.  The eight [128, cw]
exp tiles per shared chunk (4 T + 4 B m-blocks) are tree-added on DVE into
comb[128, 4096] (DMA'd out mid-kernel); the host finishes the 128-partition
column reduction, the diagonal (exact, f64), and the final mean.

Features are quantized host-side: stationary = e4m3(img * scale/16),
moving = e4m3(txt * 16), so psum = scale * <img, txt> directly.  The moving
tensor is staged in DRAM as one tensor PER CHUNK shaped like its SBUF tile,
so every partition's chunk data is a single contiguous KC*cw-byte run and
the DMA moves ~8 KB packets.
"""

import numpy as np
import ml_dtypes

import concourse.tile as tile
from concourse import bacc, mybir
from concourse.bass_utils import run_bass_kernel_spmd

TWO_N = 8192   # total rows (and T logits columns)
N = 4096       # CE rows
D = 512        # embedding dim
C = 8          # cores
R = N // C     # CE rows per core = 512
KC = D // 128  # contraction k-tiles = 4
W = 2048       # widest column chunk (psum tile width)
MB = R // 128  # 128-row blocks per core = 4
QSCALE = 16.0  # feature pre-scale before e4m3 quantization

# Schraudolph exp2 bit-trick constants (bf16 target):
#   i16 = round(x * 128*log2(e) + EXP_B); bitcast(i16) ~= exp(x).
# EXP_B = 127*128 - 7.33: the -7.33 makes E[ln(approx/true)] = 0 over a
# uniform mantissa-fraction distribution (the +4%/-4.5% piecewise-linear
# error then cancels in large sums).
EXP_A = 184.66496523378733   # 128 * log2(e)
EXP_B = 16248.9

BF16 = mybir.dt.bfloat16
FP8 = mybir.dt.float8e4
F32 = mybir.dt.float32
I16 = mybir.dt.int16

_CACHE = {}

# Uniform 2048-wide column chunks: T = 4 chunks (8192 cols), B = the first
# 2 chunk ranges (4096 cols).  Uniform width matters: a psum tile's drain
# (ACT exp ~2.0us+READ_ACC / DVE bit-exp ~2.26us) must be covered by the
# OTHER tile's matmul fill (1.73us at 2048) with psum bufs=2, so pairing a
# 2048 drain against a narrower tile's fill stalls the PE every pair.
T_CHUNKS = [2048, 2048, 2048, 2048]
NSH = 2                # chunks shared with B
PCOLS = len(T_CHUNKS)  # partials free columns (T row-sum pieces per m)


def _build():
    """Build the (core-uniform) Bass/Tile program once."""
    nc = bacc.Bacc("TRN2", target_bir_lowering=False, debug=False, num_devices=C)

    # stat1 split: the m0 slice is its own (contiguous) tensor so a tiny
    # 32KB DMA on the sync ring unblocks the very first matmul ~2us before
    # the full stationary lands on the slower gpsimd ring
    stat1a = nc.dram_tensor("stat1a", [128, KC, 128], FP8, kind="ExternalInput").ap()
    stat1b = nc.dram_tensor("stat1b", [128, KC, R - 128], FP8, kind="ExternalInput").ap()
    stat2 = nc.dram_tensor("stat2", [128, KC, R], FP8, kind="ExternalInput").ap()
    movs = [
        nc.dram_tensor(f"mov{ci}", [128, KC, cw], FP8, kind="ExternalInput").ap()
        for ci, cw in enumerate(T_CHUNKS)
    ]
    out = nc.dram_tensor("out", [128, MB, PCOLS], F32, kind="ExternalOutput").ap()
    outc = nc.dram_tensor("outc", [128, N], BF16, kind="ExternalOutput").ap()

    DR = mybir.MatmulPerfMode.DoubleRow
    EXP = mybir.ActivationFunctionType.Exp
    add = mybir.AluOpType.add
    mult = mybir.AluOpType.mult

    t_off = [0]
    for cw in T_CHUNKS:
        t_off.append(t_off[-1] + cw)

    with tile.TileContext(nc) as tc:
        with (
            tc.tile_pool(name="stat", bufs=1) as stat_pool,
            tc.tile_pool(name="acc", bufs=1) as acc_pool,
            tc.tile_pool(name="mov", bufs=1) as mov_pool,
            tc.tile_pool(name="exp", bufs=1) as exp_pool,
            tc.tile_pool(name="red", bufs=1) as red_pool,
            tc.tile_pool(name="psum", bufs=2, space="PSUM") as psum_pool,
        ):
            st1a = stat_pool.tile([128, KC, 128], FP8, tag="st1a")
            st1b = stat_pool.tile([128, KC, R - 128], FP8, tag="st1b")
            st2 = stat_pool.tile([128, KC, R], FP8, tag="st2")

            # PE warm-up: throwaway matmuls with no DMA deps start the
            # HAM clock ramp (1.2 -> 2.4 GHz) while the first chunk streams
            # (~4.5us of cold-rate matmuls covers the 1MB chunk0 transfer).
            warm = stat_pool.tile([128, 512], BF16, tag="warm")
            nc.vector.memset(warm[:], 0.0)
            wps = psum_pool.tile([128, W], F32, tag="ps")
            for _ in range(8):
                nc.tensor.matmul(
                    wps[:, 0:512], warm[:, 0:128], warm[:, 0:512],
                    start=True, stop=True,
                )

            # partials[p, m, i] = sum_j exp(T[m-block row p, chunk i cols j])
            partials = acc_pool.tile([128, MB, PCOLS], F32, tag="partials")
            nc.vector.memset(partials[:], 0.0)

            # st1's m0 slice goes FIRST on the sync ring (ahead of chunk0);
            # stat1b and stat2 ride the gpsimd ring (stat1b first: it is
            # needed at T(c0,m1) ~1us after the first matmul; stat2 is not
            # needed until the B(c0) tiles a chunk later)
            nc.sync.dma_start(st1a[:], stat1a[:])
            nc.gpsimd.dma_start(st1b[:], stat1b[:])
            nc.gpsimd.dma_start(st2[:], stat2[:])

            mov_tiles = {}   # ci -> (tile, cw) for shared chunks (kept for B)
            exp_t = {}       # (region, ci, m) -> exp tile (bf16 / int16)

            def mm_block(ps, st, blk, mt, cw):
                # st/blk: stationary tile and its local 128-row block index
                for so in range(0, cw, 512):
                    sw = min(512, cw - so)
                    for k in (0, 2):
                        nc.tensor.matmul(
                            ps[:, so:so + sw],
                            st[:, k:k + 2, blk * 128:(blk + 1) * 128],
                            mt[:, k:k + 2, so:so + sw],
                            start=(k == 0),
                            stop=(k == 2),
                            perf_mode=DR,
                        )

            def st1_of(m):
                return (st1a, 0) if m == 0 else (st1b, m - 1)

            def t_tile(ci, m):
                """T-region psum tile (ci, m): matmuls + ACT exp/row-sum."""
                cw = T_CHUNKS[ci]
                ps = psum_pool.tile([128, W], F32, tag="ps")
                stm, blk = st1_of(m)
                mm_block(ps, stm, blk, mov_tiles[ci][0], cw)
                if ci < NSH:
                    xt = exp_pool.tile([128, cw], BF16, tag=f"xT{ci}_{m}")
                    exp_t[("T", ci, m)] = xt
                    out_ap = xt[:, 0:cw]
                else:
                    out_ap = ps[:, 0:cw]   # tail: exp in place, rowsum only
                nc.scalar.activation(
                    out_ap, ps[:, 0:cw], EXP, bias=0.0,
                    accum_out=partials[:, m, ci:ci + 1],
                )

            def b_tile(ci, m):
                """B-region psum tile (ci, m): matmuls + DVE bit-trick exp."""
                cw = T_CHUNKS[ci]
                ps = psum_pool.tile([128, W], F32, tag="ps")
                mm_block(ps, st2, m, mov_tiles[ci][0], cw)
                xb = exp_pool.tile([128, cw], I16, tag=f"xB{ci}_{m}")
                exp_t[("B", ci, m)] = xb
                nc.vector.tensor_scalar(
                    xb[:, 0:cw], ps[:, 0:cw], EXP_A, EXP_B, op0=mult, op1=add,
                )

            def tree_adds(ci):
                """Column-sum add-tree steps for shared chunk ci (7 DVE adds
                + 1 outc DMA), returned as thunks so the adds can be spread
                one-per-pair between the psum-critical B-exp drains."""
                cw = T_CHUNKS[ci]
                t0, t1, t2, t3 = (exp_t[("T", ci, m)] for m in range(MB))
                b0, b1, b2, b3 = (
                    exp_t[("B", ci, m)][:].bitcast(BF16) for m in range(MB)
                )
                a01 = red_pool.tile([128, cw], BF16, tag=f"a01_{ci}")
                a23 = red_pool.tile([128, cw], BF16, tag=f"a23_{ci}")
                b01 = red_pool.tile([128, cw], BF16, tag=f"b01_{ci}")
                b23 = red_pool.tile([128, cw], BF16, tag=f"b23_{ci}")
                comb = red_pool.tile([128, cw], BF16, tag=f"cb_{ci}")
                tt = nc.vector.tensor_tensor
                return [
                    lambda: tt(out=a01[:], in0=t0[:], in1=t1[:], op=add),
                    lambda: tt(out=a23[:], in0=t2[:], in1=t3[:], op=add),
                    lambda: tt(out=a01[:], in0=a01[:], in1=a23[:], op=add),
                    lambda: tt(out=b01[:], in0=b0, in1=b1, op=add),
                    lambda: tt(out=b23[:], in0=b2, in1=b3, op=add),
                    lambda: tt(out=b01[:], in0=b01[:], in1=b23[:], op=add),
                    lambda: tt(out=comb[:], in0=a01[:], in1=b01[:], op=add),
                    lambda: nc.gpsimd.dma_start(
                        outc[:, t_off[ci]:t_off[ci] + cw], comb[:]
                    ),
                ]

            def load_mov(ci):
                cw = T_CHUNKS[ci]
                mt = mov_pool.tile([128, KC, cw], FP8, tag=f"ms{ci}")
                mov_tiles[ci] = (mt, cw)
                # all mov chunks stream FIFO on the sync ring (~190 GB/s,
                # stays ahead of the PE; splitting across rings regressed)
                nc.sync.dma_start(mt[:, :, 0:cw], movs[ci][:])

            for ci in range(PCOLS):
                load_mov(ci)

            # Schedule, phase by phase (all tiles 2048 wide):
            #   P0: T(c0) x4        solo (chunk0 DMA + HAM-warm window)
            #   P1: T(c1,m)+B(c0,m) pairs -- ACT drains the T psum while DVE
            #                       drains the B psum, both under the pair's
            #                       3.46us of matmul fill
            #   P2: T(c2,m)+B(c1,m) pairs, + chunk-0 tree adds spread one
            #                       per pair so DVE's psum drains never queue
            #   P3: T(c3) x4        solo; DVE finishes the remaining trees
            pend = []   # deferred tree-add thunks
            for m in range(MB):
                t_tile(0, m)
            for m in range(MB):
                t_tile(1, m)
                b_tile(0, m)
            pend += tree_adds(0)
            for m in range(MB):
                t_tile(2, m)
                b_tile(1, m)
                if pend:
                    pend.pop(0)()
            pend += tree_adds(1)
            for m in range(MB):
                t_tile(3, m)
                pend.pop(0)()
            while pend:
                pend.pop(0)()

            # T row-sum partials out on the sync ring (tiny, 80B/partition)
            nc.sync.dma_start(out[:], partials[:])

    nc.compile()
    return nc


def _get_nc():
    if "nc" not in _CACHE:
        _CACHE["nc"] = _build()
    return _CACHE["nc"]


def _prep_inputs(image_features, text_features, logit_scale):
    img = np.asarray(image_features, dtype=np.float32)
    txt = np.asarray(text_features, dtype=np.float32)
    scale = float(np.asarray(logit_scale, dtype=np.float32))

    # mov{ci}[p, k, j] = QSCALE * txt[t_off[ci] + j, k*128 + p], e4m3
    a = np.ascontiguousarray(txt.T * np.float32(QSCALE)).reshape(KC, 128, TWO_N)
    movf = a.transpose(1, 0, 2).astype(ml_dtypes.float8_e4m3)  # [128, KC, 2N]
    mov_chunks = {}
    off = 0
    for ci, cw in enumerate(T_CHUNKS):
        mov_chunks[f"mov{ci}"] = np.ascontiguousarray(movf[:, :, off:off + cw])
        off += cw

    def stat_layout(rows):
        # [p, k, m] = (scale/QSCALE) * feat[row0 + m, k*128 + p], e4m3
        a = (rows * np.float32(scale / QSCALE)).T.reshape(KC, 128, R)
        return np.ascontiguousarray(a.transpose(1, 0, 2).astype(ml_dtypes.float8_e4m3))

    def stat1_split(c):
        full = stat_layout(img[c * R:(c + 1) * R])
        return (np.ascontiguousarray(full[:, :, 0:128]),
                np.ascontiguousarray(full[:, :, 128:R]))

    in_maps = []
    for c in range(C):
        s1a, s1b = stat1_split(c)
        in_maps.append({
            "stat1a": s1a,
            "stat1b": s1b,
            "stat2": stat_layout(img[N + c * R:N + (c + 1) * R]),
            **mov_chunks,
        })
    # diagonal logits (same for both CE terms): scale * <img_r, txt_r>
    diag = scale * np.sum(
        img[:N].astype(np.float64) * txt[:N].astype(np.float64), axis=1
    )
    return in_maps, diag


def _finish(results, diag):
    # image CE: S_img[c*R + m*128 + p] = sum_i out[c][p, m, i]
    s = np.stack([results[c]["out"] for c in range(C)]).astype(np.float64)
    s_img = s.sum(axis=-1)  # [c, p, m]
    rows = (
        np.arange(C)[:, None, None] * R
        + np.arange(MB)[None, None, :] * 128
        + np.arange(128)[None, :, None]
    )  # [c, p, m]
    ce_img = np.mean(np.log(s_img) - diag[rows])
    # text CE: S_txt[j] = sum_c sum_p comb[c][p, j]
    combs = np.stack([np.asarray(results[c]["outc"], dtype=np.float64) for c in range(C)])
    s_txt = combs.sum(axis=(0, 1))  # [N]
    ce_txt = np.mean(np.log(s_txt) - diag)
    return np.float32((ce_img + ce_txt) / 2.0)


def kernel(image_features, text_features, logit_scale):
    nc = _get_nc()
    in_maps, diag = _prep_inputs(image_features, text_features, logit_scale)
    res = run_bass_kernel_spmd(nc, in_maps, list(range(C)))
    return _finish(res.results, diag)


if __name__ == "__main__":
    rng = np.random.default_rng(0)
    img = rng.standard_normal((TWO_N, D), dtype=np.float32)
    txt = rng.standard_normal((TWO_N, D), dtype=np.float32)
    img /= np.linalg.norm(img, axis=-1, keepdims=True)
    txt /= np.linalg.norm(txt, axis=-1, keepdims=True)
    print(kernel(img, txt, np.float32(100.0)))
